# revision 1
# baseline (speedup 1.0000x reference)
"""Distributed Trainium2 kernel for nn_Attention_10857677324470.

Sharding: 8 NeuronCores = batch item b (4) x head-half g (2).
Each core computes, for its (item, head-group-of-4):
  qkv 1x1 conv (768 of 1536 out-channels) -> depthwise 3x3 -> linear
  attention for 4 heads -> crpe refine -> partial output channels.
The only cross-core communication is a pairwise all-gather of the 256-ch
attention output between the two cores sharing one batch item, after which
each core computes its 256 proj output channels.

All weight slicing is done host-side with numpy so the device program is a
plain dense pipeline (no grouped convs except depthwise).
"""

import numpy as np
import jax
import jax.numpy as jnp
from functools import partial

P = 2
HEADS = 8
HG = 2                 # head groups (shards per item)
HPG = HEADS // HG      # heads per group = 4
FF = 4
EPS = 1e-8
WINDOWS = [3, 5, 7]
HEAD_SPLITS = [2, 3, 3]
C = 512
CH = C // HEADS        # 64
H = W = 128
N = H * W
NCORES = 8

_AXIS_GROUPS = [[0, 1], [2, 3], [4, 5], [6, 7]]


def _head_window(h):
    # head -> (crpe index, local index within that crpe's split)
    if h < 2:
        return 0, h
    if h < 5:
        return 1, h - 2
    return 2, h - 5


def _dw3x3(t, w):
    """Depthwise 3x3 as 9 shifted multiply-adds.

    t: [C', H, W] (bf16 reads, f32 accumulation via the f32 weights),
    w: [C', 3, 3] f32.
    """
    tp = jnp.pad(t.astype(jnp.bfloat16), ((0, 0), (1, 1), (1, 1)))
    out = None
    for dy in range(3):
        for dx in range(3):
            term = w[:, dy, dx][:, None, None] * \
                jax.lax.dynamic_slice(tp, (0, dy, dx), tp.shape[:1] + (H, W))
            out = term if out is None else out + term
    return out


@partial(jax.pmap, axis_name="x")
def _device_fn(x, qkv_w, dw_w, crpe_w, crpe_b, proj_w, temp, scale):
    """Per-core computation.

    x:      [512, 128, 128]   input image for this core's batch item
    qkv_w:  [768, 512]        rows: q(256) k(256) v(256) for this core's heads
    dw_w:   [768, 3, 3]       depthwise taps for those channels
    crpe_w: [4, 64, 7, 7]     per-head crpe filter (zero-padded to 7x7)
    crpe_b: [4]               per-head crpe bias
    proj_w: [256, 512]        this core's proj output rows (full 512 in-ch)
    temp:   [4, 1, 1]         temperature for this core's heads
    scale:  [4, 1, 1]         scale for this core's heads
    """
    f32 = jnp.float32
    bf16 = jnp.bfloat16

    # qkv 1x1 conv as bf16 matmul with fp32 accumulation (x arrives bf16)
    xf = x.reshape(C, N)
    qkv = jax.lax.dot(qkv_w.astype(bf16), xf,
                      preferred_element_type=f32)       # [768, N]
    qkv = _dw3x3(qkv.reshape(768, H, W), dw_w).reshape(768, N)

    q = qkv[0:256].reshape(HPG, CH, N)
    k = qkv[256:512].reshape(HPG, CH, N)
    v = qkv[512:768].reshape(HPG, CH, N)

    # per-pixel normalizers (norm over channel axis)
    q1s = 1.0 / (jnp.sqrt((q * q).sum(axis=1, keepdims=True)) + EPS)
    k1s = 1.0 / (jnp.sqrt((k * k).sum(axis=1, keepdims=True)) + EPS)
    qr = jax.nn.relu(q)
    kr = jax.nn.relu(k)
    qr2 = qr * qr
    kr2 = kr * kr
    q4 = qr2 * qr2                                      # relu(q)^4
    k4 = kr2 * kr2
    q2s = 1.0 / (jnp.sqrt((q4 * q4).sum(axis=1, keepdims=True)) + EPS)
    k2s = 1.0 / (jnp.sqrt((k4 * k4).sum(axis=1, keepdims=True)) + EPS)

    q1 = (q * q1s).astype(bf16)                         # h c n
    k1 = (k * k1s).astype(bf16)
    q2 = (q4 * q2s).astype(bf16)
    k2 = (k4 * k2s).astype(bf16)
    vb = v.astype(bf16)

    # crpe: per head, 64ch -> 7x7 -> 1ch  (grouped conv, 4 groups)
    att = jax.lax.conv_general_dilated(
        v.reshape(1, HPG * CH, H, W), crpe_w, (1, 1), [(3, 3), (3, 3)],
        feature_group_count=HPG,
        dimension_numbers=("NCHW", "OIHW", "NCHW"))[0]
    refine = jax.nn.sigmoid(
        (att + crpe_b[:, None, None]).reshape(HPG, 1, N))   # h 1 n

    # linear attention (channel-major throughout; contraction over pixels)
    attn1 = jax.lax.dot_general(k1, vb, (((2,), (2,)), ((0,), (0,))),
                                preferred_element_type=f32)   # h c d
    attn2 = jax.lax.dot_general(k2, vb, (((2,), (2,)), ((0,), (0,))),
                                preferred_element_type=f32)
    sg = jax.nn.sigmoid(scale)                          # [4,1,1]
    vsum = v.sum(axis=2, keepdims=True)                 # h d 1
    m1 = jax.lax.dot_general(attn1.astype(bf16), q1,
                             (((1,), (1,)), ((0,), (0,))),
                             preferred_element_type=f32)      # h d n
    m2 = jax.lax.dot_general((sg * attn2).astype(bf16), q2,
                             (((1,), (1,)), ((0,), (0,))),
                             preferred_element_type=f32)
    numer = vsum + m1 + m2                              # h d n
    k1sum = k1.astype(f32).sum(axis=-1)                 # h c (f32 accumulate)
    k2sum = k2.astype(f32).sum(axis=-1) * sg[:, :, 0]   # h c, scale folded in
    s1 = jnp.einsum("hcn,hc->hn", q1, k1sum.astype(bf16),
                    preferred_element_type=f32)
    s2 = jnp.einsum("hcn,hc->hn", q2, k2sum.astype(bf16),
                    preferred_element_type=f32)
    denom = f32(N) + s1[:, None, :] + s2[:, None, :] + EPS

    out = (numer / denom) * temp + refine               # h d n
    out = out.reshape(256, N)

    # gather the other head-half from the paired core, then proj
    full = jax.lax.all_gather(out.astype(bf16), "x",
                              axis_index_groups=_AXIS_GROUPS)
    full = full.reshape(C, N)                           # [512, N]
    o = jax.lax.dot(proj_w.astype(bf16), full,
                    preferred_element_type=f32)         # [256, N]
    # return bf16: halves the D2H transfer over the slow tunnel; host
    # casts back to f32 (rounding ~0.4%, well within the error budget)
    return o.reshape(256, H, W).astype(bf16)


def _build_args(x, qkv_w, dw_w, proj_w, temperature, scale,
                crpe_w0, crpe_b0, crpe_w1, crpe_b1, crpe_w2, crpe_b2):
    x = np.asarray(x, dtype=np.float32)
    qkv_w = np.asarray(qkv_w, dtype=np.float32).reshape(3072, 512)
    dw_w = np.asarray(dw_w, dtype=np.float32).reshape(3072, 3, 3)
    proj_w = np.asarray(proj_w, dtype=np.float32).reshape(1024, 512)
    temperature = np.asarray(temperature, dtype=np.float32)
    scale = np.asarray(scale, dtype=np.float32)
    crpe_ws = [np.asarray(w, dtype=np.float32) for w in (crpe_w0, crpe_w1, crpe_w2)]
    crpe_bs = [np.asarray(b, dtype=np.float32) for b in (crpe_b0, crpe_b1, crpe_b2)]

    b = x.shape[0]
    B = b // P

    xs, qkvs, dws, crpews, crpebs, projs, temps, scales = ([] for _ in range(8))
    for core in range(NCORES):
        item = core // HG
        g = core % HG
        p = item // B                      # path of this batch item
        heads = list(range(g * HPG, (g + 1) * HPG))

        # qkv rows for path p: q block then k,v; within each, this group's heads
        base = p * 3 * C
        rows = []
        for sec in range(3):               # q, k, v sections
            lo = base + sec * C + g * HPG * CH
            rows.append(np.arange(lo, lo + HPG * CH))
        rows = np.concatenate(rows)
        qkvs.append(qkv_w[rows])
        dws.append(dw_w[rows])

        # crpe filters: pad every window to 7x7 (zero padding keeps conv exact)
        cw = np.zeros((HPG, CH, 7, 7), dtype=np.float32)
        cb = np.zeros((HPG,), dtype=np.float32)
        for j, h in enumerate(heads):
            wi, li = _head_window(h)
            hs = HEAD_SPLITS[wi]
            win = WINDOWS[wi]
            pad = (7 - win) // 2
            cw[j, :, pad:7 - pad, pad:7 - pad] = crpe_ws[wi][p * hs + li]
            cb[j] = crpe_bs[wi][p * hs + li]
        crpews.append(cw)
        crpebs.append(cb)

        projs.append(proj_w[p * C + g * 256: p * C + (g + 1) * 256])
        temps.append(temperature[p, heads])
        scales.append(scale[p, heads])
        # ship x as bf16: the device casts to bf16 before the qkv matmul
        # anyway, so this halves the dominant transfer at no numerical cost
        xs.append(x[item].astype(jnp.bfloat16))

    return [np.stack(a) for a in (xs, qkvs, dws, crpews, crpebs, projs, temps, scales)], b


def _assemble(outs, b):
    result = np.empty((b, C, H, W), dtype=np.float32)
    for core in range(NCORES):
        item, g = core // HG, core % HG
        result[item, g * 256:(g + 1) * 256] = np.asarray(outs[core], dtype=np.float32)
    return result


def kernel(**inputs):
    args, b = _build_args(**inputs)
    outs = np.asarray(_device_fn(*args))   # [8, 256, 128, 128]
    return _assemble(outs, b)



# revision 22
# speedup vs baseline: 5.0504x; 5.0504x over previous
"""Distributed Trainium2 Bass kernel for nn_Attention_10857677324470.

Sharding: 8 NeuronCores = batch item (4) x head-half (2). Each core computes
qkv 1x1 conv -> depthwise 3x3 -> focused linear attention (4 heads) -> crpe
refine -> proj PARTIAL (full 512 output rows from its 256 attn channels).
Host sums the two partials of each item — no cross-core collective.
"""

import numpy as np
import ml_dtypes
from contextlib import ExitStack

P_ = 2
HEADS = 8
HG = 2
HPG = 4                # heads per core
EPS = 1e-8
C = 512
CH = 64
H = W = 128
N = H * W              # 16384
NCORES = 8
NCHUNK = 512
STRIPE = 2048          # 16 image rows
NSTRIPES = N // STRIPE
WINDOWS = [3, 5, 7]
HEAD_SPLITS = [2, 3, 3]

BF16 = ml_dtypes.bfloat16

_STATE = {}


# ----------------------------------------------------------------------------
# host-side input prep
# ----------------------------------------------------------------------------

def _head_window(h):
    if h < 2:
        return 0, h
    if h < 5:
        return 1, h - 2
    return 2, h - 5


def _lhsT_tiles(w, mtiles, ktiles):
    """w: [M, K] -> [mtiles, ktiles, 128, 128] bf16 with lhsT[k, m] layout."""
    out = np.empty((mtiles, ktiles, 128, 128), dtype=BF16)
    for m in range(mtiles):
        for k in range(ktiles):
            out[m, k] = w[m*128:(m+1)*128, k*128:(k+1)*128].T.astype(BF16)
    return out


def build_core_inputs(core, x, qkv_w, dw_w, proj_w, temperature, scale,
                      crpe_w0, crpe_b0, crpe_w1, crpe_b1, crpe_w2, crpe_b2):
    """Slice + preprocess full inputs for one core. Returns name->np.ndarray."""
    x = np.asarray(x, np.float32)
    qkv_w = np.asarray(qkv_w, np.float32).reshape(3 * C * P_, C)
    dw_w = np.asarray(dw_w, np.float32).reshape(3 * C * P_, 9)
    proj_w = np.asarray(proj_w, np.float32).reshape(P_ * C, C)
    temperature = np.asarray(temperature, np.float32).reshape(P_, HEADS)
    scale = np.asarray(scale, np.float32).reshape(P_, HEADS)
    crpe_ws = [np.asarray(w, np.float32) for w in (crpe_w0, crpe_w1, crpe_w2)]
    crpe_bs = [np.asarray(b, np.float32) for b in (crpe_b0, crpe_b1, crpe_b2)]

    item, g = core // HG, core % HG
    B = x.shape[0] // P_
    p = item // B
    heads = list(range(g * HPG, (g + 1) * HPG))

    base = p * 3 * C
    rows = np.concatenate([np.arange(base + s * C + g * 256, base + s * C + (g + 1) * 256)
                           for s in range(3)])
    wq = qkv_w[rows]                     # [768, 512]
    wqkv = np.empty((128, 24, 128), dtype=BF16)   # [part, (m,k), mcol]
    t6 = _lhsT_tiles(wq, 6, 4)
    for m in range(6):
        for k in range(4):
            wqkv[:, m*4+k, :] = t6[m, k]
    dwt = dw_w[rows].astype(np.float32).reshape(6, 128, 9).transpose(1, 0, 2).copy()

    wp = proj_w[p*C:(p+1)*C, g*256:(g+1)*256]    # [512, 256]
    tp = _lhsT_tiles(wp, 4, 2)
    wproj = np.empty((128, 8, 128), dtype=BF16)
    for m in range(4):
        for k in range(2):
            wproj[:, m*2+k, :] = tp[m, k]

    # crpe lhsT per head-pair: [128 (2 heads x 64ch), 98 (2 heads x 49 taps)]
    # tap column m = dx*14 + j*7 + dy  (j = head index within pair)
    wcrpe = np.zeros((128, 2, 110), dtype=np.float32)
    cb = np.zeros(HPG, dtype=np.float32)
    for jh, hh in enumerate(heads):
        wi, li = _head_window(hh)
        hs = HEAD_SPLITS[wi]
        win = WINDOWS[wi]
        pad = (7 - win) // 2
        full = np.zeros((CH, 7, 7), dtype=np.float32)
        full[:, pad:7-pad, pad:7-pad] = crpe_ws[wi][p * hs + li]
        cb[jh] = crpe_bs[wi][p * hs + li]
        pair, j = jh // 2, jh % 2
        for dy in range(7):
            for dx in range(7):
                wcrpe[j*64:(j+1)*64, pair, dx*16 + j*7 + dy] = full[:, dy, dx]

    ones2 = np.zeros((128, 16), dtype=BF16)
    ones2[0:64, 0] = 1
    ones2[64:128, 1] = 1

    consts = np.zeros((1, 12), dtype=np.float32)
    consts[0, 0:4] = temperature[p, heads]
    consts[0, 4:8] = 1.0 / (1.0 + np.exp(-scale[p, heads]))
    consts[0, 8:12] = cb

    return {
        "x": x[item].reshape(4, 128, N).astype(BF16),
        "wqkv": wqkv,
        "dwt": dwt,
        "wproj": wproj,
        "wcrpe": wcrpe.astype(BF16),
        "ones2": ones2,
        "consts": consts,
    }


# ----------------------------------------------------------------------------
# device program
# ----------------------------------------------------------------------------

class Builder:
    def __init__(self, stop_after=None, pe_taps=0):
        import concourse.bass as bass
        import concourse.tile as tile
        from concourse import mybir, bacc
        self.bass, self.tile, self.mybir = bass, tile, mybir
        self.f32 = mybir.dt.float32
        self.bf16 = mybir.dt.bfloat16
        self.AF = mybir.ActivationFunctionType
        self.OP = mybir.AluOpType
        self.stop_after = stop_after
        self.pe_taps = pe_taps
        self.dbg = {}

        nc = bacc.Bacc("TRN2", target_bir_lowering=False, debug=False,
                       enable_asserts=False, num_devices=NCORES)
        self.nc = nc
        f32, bf16 = self.f32, self.bf16
        self.i_x = nc.dram_tensor("x", [4, 128, N], bf16, kind="ExternalInput").ap()
        self.i_wqkv = nc.dram_tensor("wqkv", [128, 24, 128], bf16, kind="ExternalInput").ap()
        self.i_dwt = nc.dram_tensor("dwt", [128, 6, 9], f32, kind="ExternalInput").ap()
        self.i_wproj = nc.dram_tensor("wproj", [128, 8, 128], bf16, kind="ExternalInput").ap()
        self.i_wcrpe = nc.dram_tensor("wcrpe", [128, 2, 110], bf16, kind="ExternalInput").ap()
        self.i_ones2 = nc.dram_tensor("ones2", [128, 16], bf16, kind="ExternalInput").ap()
        self.i_consts = nc.dram_tensor("consts", [1, 12], f32, kind="ExternalInput").ap()
        self.o_out = nc.dram_tensor("out", [4, 128, N], bf16, kind="ExternalOutput").ap()

    def dbg_out(self, name, shape, dtype=None):
        ap = self.nc.dram_tensor(name, shape, dtype or self.f32,
                                 kind="ExternalOutput").ap()
        self.dbg[name] = ap
        return ap

    def build(self):
        nc, tile = self.nc, self.tile
        with tile.TileContext(nc) as tc, ExitStack() as ctx:
            self.tc = tc
            self.ctx = ctx
            self.setup()
            done = self.phases()
            del self.tc, self.ctx
        nc.compile()
        return nc, self.dbg

    # ------------------------------------------------------------------
    def setup(self):
        nc, tc, ctx = self.nc, self.tc, self.ctx
        f32, bf16 = self.f32, self.bf16
        bass = self.bass

        const_p = ctx.enter_context(tc.tile_pool(name="const", bufs=1))
        persist = ctx.enter_context(tc.tile_pool(name="persist", bufs=1))
        dram_p = ctx.enter_context(tc.tile_pool(name="drams", bufs=1, space="DRAM"))
        self.const_p, self.persist, self.dram_p = const_p, persist, dram_p

        self.w_qkv = const_p.tile([128, 24, 128], bf16, tag="wqkv")
        nc.sync.dma_start(self.w_qkv[:], self.i_wqkv[:])
        self.dwt_sb = const_p.tile([128, 6, 9], f32, tag="dwt")
        nc.sync.dma_start(self.dwt_sb[:], self.i_dwt[:])
        self.w_proj = const_p.tile([128, 8, 128], bf16, tag="wproj")
        nc.sync.dma_start(self.w_proj[:], self.i_wproj[:])
        self.w_crpe = const_p.tile([128, 2, 110], bf16, tag="wcrpe")
        nc.sync.dma_start(self.w_crpe[:], self.i_wcrpe[:])
        self.ones2 = const_p.tile([128, 16], bf16, tag="ones2")
        nc.sync.dma_start(self.ones2[:], self.i_ones2[:])
        self.consts = const_p.tile([128, 12], f32, tag="consts")
        cb_ap = bass.AP(tensor=self.i_consts.tensor, offset=self.i_consts.offset,
                        ap=[[0, 128]] + [list(d) for d in self.i_consts.ap[1:]])
        nc.gpsimd.dma_start(self.consts[:], cb_ap)

        self.k4_dram = dram_p.tile([128, 2, N], bf16)
        self.q4_dram = dram_p.tile([128, 2, N], bf16)
        # rows: 0-3 m1(h), 4-7 m2'(h), 8-11 a(h), 12-15 b(h), 16-19 c(h),
        # 20-23 refine(h)
        self.rows_dram = dram_p.tile([24, N], bf16)

        self.kv = persist.tile([128, 4, N], bf16, tag="kv")
        self.vsum_col = persist.tile([128, 2, 9], f32, tag="vsum")
        # compact per-pixel tensors: [p, tile/pair, block, head] (n = b*128 + p)
        self.m1_c = persist.tile([128, 2, 128, 2], f32, tag="m1c")
        self.m2_c = persist.tile([128, 2, 128, 2], f32, tag="m2c")
        self.refine_c = persist.tile([128, 2, 128, 2], f32, tag="refc")

    # ------------------------------------------------------------------
    def qkv_dw_mtile(self, mt, dwout_ap, vsum_slot=None):
        """qkv matmul + depthwise 3x3 for one 128-channel output tile."""
        nc, tc = self.nc, self.tc
        f32, bf16 = self.f32, self.bf16
        with tc.tile_pool(name=f"mm{mt}", bufs=4, space="PSUM") as psum_p, \
             tc.tile_pool(name=f"xs{mt}", bufs=3) as x_p, \
             tc.tile_pool(name=f"hb{mt}", bufs=3) as hb_p:
            hbs = [None] * NSTRIPES
            for s in range(NSTRIPES):
                hb = hb_p.tile([128, 18, 128], bf16, tag="hb")
                hbs[s] = hb
                hbf = hb[:].rearrange("p y x -> p (y x)")
                for cch in range(STRIPE // NCHUNK):
                    ps = psum_p.tile([128, NCHUNK], f32, tag="ps")
                    n0 = s * STRIPE + cch * NCHUNK
                    xt = x_p.tile([128, 4, NCHUNK], bf16, tag="x")
                    nc.sync.dma_start(
                        xt[:], self.i_x[:, :, n0:n0+NCHUNK].rearrange("k p n -> p k n"))
                    for k in range(4):
                        nc.tensor.matmul(ps[:], self.w_qkv[:, mt*4+k], xt[:, k],
                                         start=(k == 0), stop=(k == 3))
                    hcol = 128 + cch * NCHUNK
                    nc.scalar.copy(hbf[:, hcol:hcol+NCHUNK], ps[:])
                if s == 0:
                    nc.vector.memset(hbf[:, 0:128], 0.0)
                else:
                    prev = hbs[s-1][:].rearrange("p y x -> p (y x)")
                    nc.vector.tensor_copy(hbf[:, 0:128], prev[:, 2048:2176])
                    nc.vector.tensor_copy(prev[:, 2176:2304], hbf[:, 128:256])
                    self._dw_taps(mt, s - 1, hbs[s-1], dwout_ap, vsum_slot)
                if s == NSTRIPES - 1:
                    nc.vector.memset(hbf[:, 2176:2304], 0.0)
                    self._dw_taps(mt, s, hbs[s], dwout_ap, vsum_slot)

    def _dw_taps(self, mt, s, hb, dwout_ap, vsum_slot):
        nc, OP = self.nc, self.OP
        out_s = dwout_ap[:, s*STRIPE:(s+1)*STRIPE].rearrange("p (y x) -> p y x", x=128)
        hb3 = hb[:]
        if vsum_slot is not None:
            nc.gpsimd.memset(out_s[:], 0.0)
            order = [(dy, dx) for dy in range(3) for dx in range(3)
                     if not (dy == 1 and dx == 1)] + [(1, 1)]
        else:
            order = [(1, 1)] + [(dy, dx) for dy in range(3) for dx in range(3)
                                if not (dy == 1 and dx == 1)]
        for idx, (dy, dx) in enumerate(order):
            tap = dy * 3 + dx
            w_col = self.dwt_sb[:, mt, tap:tap+1]
            if dx == 0:
                xo, xi, xn = 1, 0, 127
            elif dx == 2:
                xo, xi, xn = 0, 1, 127
            else:
                xo, xi, xn = 0, 0, 128
            o_ap = out_s[:, :, xo:xo+xn]
            i_ap = hb3[:, dy:dy+16, xi:xi+xn]
            if vsum_slot is None and idx == 0:
                nc.vector.tensor_scalar_mul(o_ap, i_ap, w_col)
            else:
                acc = None
                if vsum_slot is not None and (dy, dx) == (1, 1):
                    acc = vsum_slot[:, s:s+1]
                nc.vector.scalar_tensor_tensor(
                    o_ap, i_ap, w_col, o_ap, OP.mult, OP.add, accum_out=acc)

    # ------------------------------------------------------------------
    def phases(self):
        nc, OP = self.nc, self.OP
        # P1a: k tiles (kv 0,1), v tiles (kv 2,3)
        for j, mt in enumerate([2, 3, 4, 5]):
            vs = self.vsum_col[:, mt - 4] if mt >= 4 else None
            self.qkv_dw_mtile(mt, self.kv[:, j], vs)
        for vt in range(2):
            nc.vector.tensor_reduce(self.vsum_col[:, vt, 8:9],
                                    self.vsum_col[:, vt, 0:8],
                                    axis=self.mybir.AxisListType.X, op=OP.add)

        if self.stop_after == "p1a":
            d = self.dbg_out("dbg_kv", [128, 4, N], self.bf16)
            nc.sync.dma_start(d[:], self.kv[:])
            dv = self.dbg_out("dbg_vsum", [128, 2])
            nc.sync.dma_start(dv[:], self.vsum_col[:, :, 8])
            return

        self.p1b_kstats()
        self.p1c_crpe()
        self._maybe_p1bc_dbg()
        if self.stop_after == "p1bc":
            return
        self.p1d_attn()
        if self.stop_after == "p1d":
            d = self.dbg_out("dbg_attnsb", [128, 2, 2, 65], self.bf16)
            nc.sync.dma_start(d[:], self.attn_sb[:])
            d2 = self.dbg_out("dbg_vr", [2, 4, 64], self.bf16)
            nc.sync.dma_start(d2[:], self.vr_lhsT[:])
            d3 = self.dbg_out("dbg_ksum", [128, 2, 2, 16], self.bf16)
            nc.sync.dma_start(d3[:], self.ksum_lhsT[:])
            return
        self.p2_q()
        if self.stop_after == "p2":
            d = self.dbg_out("dbg_attnout", [128, 2, N], self.bf16)
            nc.sync.dma_start(d[:], self.qa[:, 2:4])
            d2 = self.dbg_out("dbg_rows", [24, N], self.bf16)
            nc.sync.dma_start(d2[:], self.rows_dram[:])
            return
        self.p3_proj()
        return

    def _maybe_p1bc_dbg(self):
        nc = self.nc
        if self.stop_after == "p1bc":
            d1 = self.dbg_out("dbg_m1c", [128, 2, 128, 2])
            nc.sync.dma_start(d1[:], self.m1_c[:])
            d2 = self.dbg_out("dbg_m2c", [128, 2, 128, 2])
            nc.sync.dma_start(d2[:], self.m2_c[:])
            d3 = self.dbg_out("dbg_refc", [128, 2, 128, 2])
            nc.sync.dma_start(d3[:], self.refine_c[:])
            d4 = self.dbg_out("dbg_k4", [128, 2, N], self.bf16)
            nc.sync.dma_start(d4[:], self.k4_dram[:])
            d5 = self.dbg_out("dbg_mrows", [24, N], self.bf16)
            nc.sync.dma_start(d5[:], self.rows_dram[:])
            return
        return

    # ------------------------------------------------------------------
    def p1b_kstats(self):
        nc, tc, OP, AF = self.nc, self.tc, self.OP, self.AF
        f32, bf16 = self.f32, self.bf16
        AX = self.mybir.AxisListType
        with tc.tile_pool(name="kst_ps", bufs=1, space="PSUM") as pp, \
             tc.tile_pool(name="kst_sb", bufs=3) as sp, \
             tc.tile_pool(name="kst_scr", bufs=1) as scp:
            scr1 = scp.tile([128, 2, 4, 32, 16], bf16, tag="scr1")
            scr2 = scp.tile([128, 2, 4, 32, 16], bf16, tag="scr2")
            for kt in range(2):
                for sc in range(4):
                    ps = pp.tile([128, 4096], f32, tag="ps")
                    for cc in range(8):
                        n0 = sc * 4096 + cc * NCHUNK
                        kch = self.kv[:, kt, n0:n0+NCHUNK]
                        ksq = sp.tile([128, NCHUNK], bf16, tag="ksq")
                        nc.vector.tensor_mul(ksq[:], kch, kch)
                        r = sp.tile([128, NCHUNK], bf16, tag="r")
                        nc.scalar.activation(r[:], kch, AF.Relu)
                        r2 = sp.tile([128, NCHUNK], bf16, tag="r2")
                        nc.vector.tensor_mul(r2[:], r[:], r[:])
                        k4 = sp.tile([128, NCHUNK], bf16, tag="k4")
                        nc.vector.tensor_mul(k4[:], r2[:], r2[:])
                        nc.sync.dma_start(self.k4_dram[:, kt, n0:n0+NCHUNK], k4[:])
                        k42 = sp.tile([128, NCHUNK], bf16, tag="k42")
                        nc.vector.tensor_mul(k42[:], k4[:], k4[:])
                        nc.tensor.matmul(ps[0:16, cc*NCHUNK:(cc+1)*NCHUNK],
                                         self.ones2[:], ksq[:], start=True, stop=True)
                        nc.tensor.matmul(ps[32:48, cc*NCHUNK:(cc+1)*NCHUNK],
                                         self.ones2[:], k42[:], start=True, stop=True)
                    sbf = sp.tile([128, 4096], bf16, tag="sbf")
                    nc.scalar.copy(sbf[0:16, :], ps[0:16, :])
                    nc.scalar.copy(sbf[32:48, :], ps[32:48, :])
                    nc.sync.dma_start_transpose(scr1[:, kt, sc], sbf[0:16, :])
                    nc.sync.dma_start_transpose(scr2[:, kt, sc], sbf[32:48, :])
            # row math -> m1_c, m2_c
            for kt in range(2):
                nn1 = scr1[:, kt].rearrange("p s b r -> p (s b) r")[:, :, 0:2]
                nn2 = scr2[:, kt].rearrange("p s b r -> p (s b) r")[:, :, 0:2]
                m1 = self.m1_c[:, kt]
                m2 = self.m2_c[:, kt]
                nc.scalar.activation(m1[:], nn1, AF.Sqrt)
                nc.vector.tensor_scalar_add(m1[:], m1[:], EPS)
                nc.vector.reciprocal(m1[:], m1[:])
                nc.scalar.activation(m2[:], nn2, AF.Sqrt)
                nc.vector.tensor_scalar_add(m2[:], m2[:], EPS)
                nc.vector.reciprocal(m2[:], m2[:])
                for hh in range(2):
                    nc.vector.tensor_scalar_mul(m2[:, :, hh:hh+1], m2[:, :, hh:hh+1],
                                                self.consts[:, 4+kt*2+hh:5+kt*2+hh])
            # bf16 + rowify to rows_dram rows 0-3 (m1), 4-7 (m2)
            with tc.tile_pool(name="rowify", bufs=2) as rp:
                for ti, (src, row0) in enumerate([(self.m1_c, 0), (self.m2_c, 4)]):
                    for kt in range(2):
                        for hh in range(2):
                            mbf = rp.tile([128, 128], bf16, tag="mbf")
                            nc.vector.tensor_copy(mbf[:], src[:, kt, :, hh])
                            tr = rp.tile([128, 128], bf16, tag="tr")
                            nc.sync.dma_start_transpose(tr[:], mbf[:])
                            nc.sync.dma_start(
                                self.rows_dram[row0+kt*2+hh].rearrange("(b p) -> b p", p=128),
                                tr[:])

    # ------------------------------------------------------------------
    def p1c_crpe(self):
        nc, tc, OP, AF = self.nc, self.tc, self.OP, self.AF
        f32, bf16 = self.f32, self.bf16
        taps_dram = self.dram_p.tile([112, N], bf16)
        with tc.tile_pool(name="crpe_ps", bufs=2, space="PSUM") as pp, \
             tc.tile_pool(name="crpe_sb", bufs=1) as sp, \
             tc.tile_pool(name="crpe_sc", bufs=3) as scp, \
             tc.tile_pool(name="crpe_sm", bufs=2) as smp:
            acc16 = sp.tile([16, N], bf16, tag="acc16")
            att16 = sp.tile([128, 128, 16], bf16, tag="att16")
            for pair in range(2):
                for cc in range(32):
                    ps = pp.tile([128, NCHUNK], f32, tag="ps")
                    nc.tensor.matmul(ps[0:110, :], self.w_crpe[:, pair, 0:110],
                                     self.kv[:, 2+pair, cc*NCHUNK:(cc+1)*NCHUNK],
                                     start=True, stop=True)
                    tch = scp.tile([112, NCHUNK], bf16, tag="tch")
                    nc.scalar.copy(tch[0:110, :], ps[0:110, :])
                    nc.sync.dma_start(taps_dram[0:110, cc*NCHUNK:(cc+1)*NCHUNK],
                                      tch[0:110, :])
                tap3 = taps_dram[:].rearrange("q (y x) -> q y x", x=128)
                ac3 = acc16[:].rearrange("q (y x) -> q y x", x=128)
                for dx in range(7):
                    sh = dx - 3
                    xo = max(0, -sh)
                    xn = 128 - abs(sh)
                    src = tap3[dx*16:(dx+1)*16, :, xo+sh:xo+sh+xn]
                    dst = ac3[:, :, xo:xo+xn]
                    if dx == 3:
                        nc.gpsimd.dma_start(ac3[:, :, :], tap3[48:64, :, :])
                    else:
                        nc.gpsimd.dma_start(dst, src, accum_op=OP.add)
                nc.sync.dma_start_transpose(att16[:], acc16[:])
                for hh in range(2):
                    attf = smp.tile([128, 128], f32, tag="attf")
                    nc.vector.tensor_copy(attf[:], att16[:, :, hh*7+3])
                    for dy in (0, 1, 2, 4, 5, 6):
                        sh = dy - 3
                        bo = max(0, -sh)
                        bn = 128 - abs(sh)
                        nc.vector.tensor_add(attf[:, bo:bo+bn],
                                             att16[:, bo+sh:bo+sh+bn, hh*7+dy],
                                             attf[:, bo:bo+bn])
                    nc.scalar.activation(self.refine_c[:, pair, :, hh], attf[:],
                                         AF.Sigmoid,
                                         bias=self.consts[:, 8+pair*2+hh:9+pair*2+hh])

    # ------------------------------------------------------------------
    def p1d_attn(self):
        nc, tc, OP = self.nc, self.tc, self.OP
        f32, bf16 = self.f32, self.bf16
        bass = self.bass
        persist = self.persist
        TS = 1024              # transpose stripe (8 pixel blocks)
        NB = TS // 128
        NS = N // TS

        self.attn_sb = persist.tile([128, 2, 2, 65], bf16, tag="attnsb")
        self.ksum_lhsT = persist.tile([128, 2, 2, 16], bf16, tag="ksuml")
        self.vr_lhsT = persist.tile([2, 4, 64], bf16, tag="vrl")

        with tc.tile_pool(name="att_ps", bufs=1, space="PSUM") as pp, \
             tc.tile_pool(name="att_bc", bufs=2) as bp, \
             tc.tile_pool(name="att_tp", bufs=2) as tp, \
             tc.tile_pool(name="att_f", bufs=2) as fp:
            aps00 = pp.tile([128, 65], f32, tag="aps00")
            aps01 = pp.tile([128, 65], f32, tag="aps01")
            aps02 = pp.tile([128, 65], f32, tag="aps02")
            aps03 = pp.tile([128, 65], f32, tag="aps03")
            aps10 = pp.tile([128, 65], f32, tag="aps10")
            aps11 = pp.tile([128, 65], f32, tag="aps11")
            aps12 = pp.tile([128, 65], f32, tag="aps12")
            aps13 = pp.tile([128, 65], f32, tag="aps13")
            att_ps = [[aps00, aps01, aps02, aps03],
                      [aps10, aps11, aps12, aps13]]
            for vt in range(2):
                for s in range(NS):
                    n0 = s * TS
                    m1bc = bp.tile([128, TS], bf16, tag="m1bc")
                    m2bc = bp.tile([128, TS], bf16, tag="m2bc")
                    for hh in range(2):
                        for (bcast, row0) in ((m1bc, 0), (m2bc, 4)):
                            src = self.rows_dram[row0+vt*2+hh, n0:n0+TS]
                            ap = bass.AP(tensor=src.tensor, offset=src.offset,
                                         ap=[[0, 64]] + [list(d) for d in src.ap])
                            nc.gpsimd.dma_start(bcast[hh*64:(hh+1)*64, :], ap)
                    v1s = bp.tile([128, TS], bf16, tag="v1s")
                    v2s = bp.tile([128, TS], bf16, tag="v2s")
                    vstripe = self.kv[:, 2+vt, n0:n0+TS]
                    nc.vector.tensor_mul(v1s[:], vstripe, m1bc[:])
                    nc.vector.tensor_mul(v2s[:], vstripe, m2bc[:])
                    t_v1 = tp.tile([128, NB, 128], bf16, tag="tv1")
                    nc.sync.dma_start_transpose(t_v1[:], v1s[:])
                    t_v2 = tp.tile([128, NB, 128], bf16, tag="tv2")
                    nc.sync.dma_start_transpose(t_v2[:], v2s[:])
                    t_k = tp.tile([128, NB, 128], bf16, tag="tk")
                    nc.sync.dma_start_transpose(t_k[:], self.kv[:, vt, n0:n0+TS])
                    t_k4 = tp.tile([128, NB, 128], bf16, tag="tk4")
                    nc.sync.dma_start_transpose(t_k4[:], self.k4_dram[:, vt, n0:n0+TS])
                    kk4 = tp.tile([128, NB, 256], bf16, tag="kk4")
                    vB = tp.tile([128, NB, 260], bf16, tag="vB")
                    for hh in range(2):
                        nc.vector.tensor_copy(kk4[:, :, hh*128:hh*128+64],
                                              t_k[:, :, hh*64:(hh+1)*64])
                        nc.vector.tensor_copy(kk4[:, :, hh*128+64:hh*128+128],
                                              t_k4[:, :, hh*64:(hh+1)*64])
                        q1 = (hh*2) * 65
                        q2 = (hh*2+1) * 65
                        nc.vector.tensor_copy(vB[:, :, q1:q1+64],
                                              t_v1[:, :, hh*64:(hh+1)*64])
                        nc.vector.tensor_copy(vB[:, :, q1+64:q1+65],
                                              self.m1_c[:, vt, s*NB:(s+1)*NB, hh:hh+1])
                        nc.vector.tensor_copy(vB[:, :, q2:q2+64],
                                              t_v2[:, :, hh*64:(hh+1)*64])
                        nc.vector.tensor_copy(vB[:, :, q2+64:q2+65],
                                              self.m2_c[:, vt, s*NB:(s+1)*NB, hh:hh+1])
                    for b in range(NB):
                        for hh in range(2):
                            for t_ in range(2):
                                q = hh*2 + t_
                                lhsT = kk4[:, b, hh*128+t_*64:hh*128+t_*64+64]
                                nc.tensor.matmul(
                                    att_ps[vt][q][hh*64:(hh+1)*64, :],
                                    lhsT, vB[:, b, q*65:(q+1)*65],
                                    start=(s == 0 and b == 0),
                                    stop=(s == NS-1 and b == NB-1),
                                    tile_position=(0, hh*64))
            # extract: psum rows already at target partitions; cast on ACT
            for vt in range(2):
                for hh in range(2):
                    for t_ in range(2):
                        q = hh*2 + t_
                        src = att_ps[vt][q][hh*64:(hh+1)*64, :]
                        nc.scalar.copy(self.attn_sb[hh*64:(hh+1)*64, vt, t_], src)
            # ksum lhsT columns (zero-padded)
            nc.vector.memset(self.ksum_lhsT[:], 0.0)
            for vt in range(2):
                for hh in range(2):
                    for t_ in range(2):
                        nc.vector.tensor_copy(
                            self.ksum_lhsT[hh*64:(hh+1)*64, vt, t_, hh:hh+1],
                            self.attn_sb[hh*64:(hh+1)*64, vt, t_, 64:65])
            # vsum-refine lhsT: row 0 vsum_h (DMA overwrites), row 1 ones
            nc.vector.memset(self.vr_lhsT[:], 1.0)
            vs_bf = fp.tile([128, 2], bf16, tag="vsbf")
            nc.vector.tensor_copy(vs_bf[:], self.vsum_col[:, :, 8])
            for vt in range(2):
                for hh in range(2):
                    nc.sync.dma_start(self.vr_lhsT[0:1, vt*2+hh, :],
                                      vs_bf[hh*64:(hh+1)*64, vt:vt+1])

    # ------------------------------------------------------------------
    def p2_q(self):
        nc, tc, OP, AF = self.nc, self.tc, self.OP, self.AF
        f32, bf16 = self.f32, self.bf16
        bass = self.bass
        AX = self.mybir.AxisListType

        # q tiles and attnout reuse kv's buffer (tag="kv")
        self.qa = self.persist.tile([128, 4, N], bf16, tag="kv")
        for mt in range(2):
            self.qkv_dw_mtile(mt, self.qa[:, mt])

        # ---- q stats + row math, per q-tile ----
        with tc.tile_pool(name="qst_ps", bufs=1, space="PSUM") as pp, \
             tc.tile_pool(name="qst_sb", bufs=2) as sp, \
             tc.tile_pool(name="qst_sbf", bufs=2) as sbp, \
             tc.tile_pool(name="qst_scr", bufs=1) as scp, \
             tc.tile_pool(name="qrow", bufs=1) as rp, \
             tc.tile_pool(name="qrow2", bufs=2) as rp2:
            for t in range(2):
                qscr0 = scp.tile([128, 4, 32, 16], bf16, tag="qscr0")
                qscr1 = scp.tile([128, 4, 32, 16], bf16, tag="qscr1")
                qscr2 = scp.tile([128, 4, 32, 16], bf16, tag="qscr2")
                qscr3 = scp.tile([128, 4, 32, 16], bf16, tag="qscr3")
                scrs = [qscr0, qscr1, qscr2, qscr3]
                for sc in range(4):
                    ps = pp.tile([128, 4096], f32, tag="ps")
                    for cc in range(8):
                        n0 = sc * 4096 + cc * NCHUNK
                        qch = self.qa[:, t, n0:n0+NCHUNK]
                        qsq = sp.tile([128, NCHUNK], bf16, tag="qsq")
                        nc.vector.tensor_mul(qsq[:], qch, qch)
                        r = sp.tile([128, NCHUNK], bf16, tag="r")
                        nc.scalar.activation(r[:], qch, AF.Relu)
                        r2 = sp.tile([128, NCHUNK], bf16, tag="r2")
                        nc.vector.tensor_mul(r2[:], r[:], r[:])
                        q4 = sp.tile([128, NCHUNK], bf16, tag="q4")
                        nc.vector.tensor_mul(q4[:], r2[:], r2[:])
                        nc.sync.dma_start(self.q4_dram[:, t, n0:n0+NCHUNK], q4[:])
                        q42 = sp.tile([128, NCHUNK], bf16, tag="q42")
                        nc.vector.tensor_mul(q42[:], q4[:], q4[:])
                        cs = slice(cc*NCHUNK, (cc+1)*NCHUNK)
                        nc.tensor.matmul(ps[0:16, cs], self.ones2[:], qsq[:],
                                         start=True, stop=True,
                                         tile_position=(0, 0))
                        nc.tensor.matmul(ps[32:48, cs], self.ones2[:], q42[:],
                                         start=True, stop=True,
                                         tile_position=(0, 32))
                        nc.tensor.matmul(ps[64:80, cs], self.ksum_lhsT[:, t, 0],
                                         qch, start=True, stop=True,
                                         tile_position=(0, 64))
                        nc.tensor.matmul(ps[96:112, cs], self.ksum_lhsT[:, t, 1],
                                         q4[:], start=True, stop=True,
                                         tile_position=(0, 96))
                    sbf = sbp.tile([128, 4096], bf16, tag="sbf")
                    for g in range(4):
                        nc.scalar.copy(sbf[g*32:g*32+16, :], ps[g*32:g*32+16, :])
                        nc.sync.dma_start_transpose(scrs[g][:, sc],
                                                    sbf[g*32:g*32+16, :])
                # ---- row math (compact, f32) ----
                view = [sc_[:].rearrange("p s b r -> p (s b) r")[:, :, 0:2]
                        for sc_ in scrs]
                nn1, nn2, sr1, sr2 = view
                n1 = rp.tile([128, 128, 2], f32, tag="n1")
                n2 = rp.tile([128, 128, 2], f32, tag="n2")
                den = rp.tile([128, 128, 2], f32, tag="den")
                tden = rp.tile([128, 128, 2], f32, tag="tden")
                nc.scalar.activation(n1[:], nn1, AF.Sqrt)
                nc.vector.tensor_scalar_add(n1[:], n1[:], EPS)
                nc.vector.reciprocal(n1[:], n1[:])
                nc.scalar.activation(n2[:], nn2, AF.Sqrt)
                nc.vector.tensor_scalar_add(n2[:], n2[:], EPS)
                nc.vector.reciprocal(n2[:], n2[:])
                nc.vector.tensor_mul(den[:], n1[:], sr1)
                tmp = rp2.tile([128, 128, 2], f32, tag="tmp")
                nc.vector.tensor_mul(tmp[:], n2[:], sr2)
                nc.vector.tensor_add(den[:], den[:], tmp[:])
                nc.vector.tensor_scalar_add(den[:], den[:], float(N) + EPS)
                nc.vector.reciprocal(den[:], den[:])
                for hh in range(2):
                    h = 2*t + hh
                    nc.vector.tensor_scalar_mul(
                        tden[:, :, hh:hh+1], den[:, :, hh:hh+1],
                        self.consts[:, h:h+1])
                a_c = rp2.tile([128, 128, 2], f32, tag="a_c")
                b_c = rp2.tile([128, 128, 2], f32, tag="b_c")
                nc.vector.tensor_mul(a_c[:], n1[:], tden[:])
                nc.vector.tensor_mul(b_c[:], n2[:], tden[:])
                for (src, row0) in ((a_c, 8), (b_c, 12), (tden, 16)):
                    for hh in range(2):
                        mbf = rp2.tile([128, 128], bf16, tag="mbf")
                        nc.vector.tensor_copy(mbf[:], src[:, :, hh])
                        tr = rp2.tile([128, 128], bf16, tag="tr")
                        nc.sync.dma_start_transpose(tr[:], mbf[:])
                        nc.sync.dma_start(
                            self.rows_dram[row0+t*2+hh].rearrange(
                                "(b p) -> b p", p=128), tr[:])
                for hh in range(2):
                    mbf = rp2.tile([128, 128], bf16, tag="mbf")
                    nc.vector.tensor_copy(mbf[:], self.refine_c[:, t, :, hh])
                    tr = rp2.tile([128, 128], bf16, tag="tr")
                    nc.sync.dma_start_transpose(tr[:], mbf[:])
                    nc.sync.dma_start(
                        self.rows_dram[20+t*2+hh].rearrange(
                            "(b p) -> b p", p=128), tr[:])

        # ---- finale: M1 + M2 + (vsum*c + refine) ----
        with tc.tile_pool(name="fin_ps", bufs=4, space="PSUM") as pp, \
             tc.tile_pool(name="fin_sb", bufs=3) as bp:
            for t in range(2):
                for cc in range(32):
                    n0 = cc * NCHUNK
                    abch = bp.tile([128, 2, NCHUNK], bf16, tag="abch")
                    for hh in range(2):
                        for ti, row0 in ((0, 8), (1, 12)):
                            src = self.rows_dram[row0+t*2+hh, n0:n0+NCHUNK]
                            ap = bass.AP(tensor=src.tensor, offset=src.offset,
                                         ap=[[0, 64]] + [list(d) for d in src.ap])
                            nc.gpsimd.dma_start(abch[hh*64:(hh+1)*64, ti, :], ap)
                    q4ch = bp.tile([128, NCHUNK], bf16, tag="q4ch")
                    nc.sync.dma_start(q4ch[:], self.q4_dram[:, t, n0:n0+NCHUNK])
                    qt1 = bp.tile([128, NCHUNK], bf16, tag="qt1")
                    qt2 = bp.tile([128, NCHUNK], bf16, tag="qt2")
                    nc.vector.tensor_mul(qt1[:], self.qa[:, t, n0:n0+NCHUNK],
                                         abch[:, 0])
                    nc.vector.tensor_mul(qt2[:], q4ch[:], abch[:, 1])
                    crch = bp.tile([2, 4, NCHUNK], bf16, tag="crch")
                    src = self.rows_dram[16:24, n0:n0+NCHUNK]
                    ap = bass.AP(tensor=src.tensor, offset=src.offset,
                                 ap=[[4 * N, 2], [N, 4], [1, NCHUNK]])
                    nc.gpsimd.dma_start(crch[:], ap)
                    ps = pp.tile([128, NCHUNK], f32, tag="ps")
                    for hh in range(2):
                        h = 2*t + hh
                        sl = slice(hh*64, (hh+1)*64)
                        tp_ = (hh*64, hh*64)
                        nc.tensor.matmul(ps[sl, :],
                                         self.attn_sb[sl, t, 0, 0:64],
                                         qt1[sl, :], start=True, stop=False,
                                         tile_position=tp_)
                        nc.tensor.matmul(ps[sl, :],
                                         self.attn_sb[sl, t, 1, 0:64],
                                         qt2[sl, :], start=False, stop=False,
                                         tile_position=tp_)
                        nc.tensor.matmul(ps[sl, :],
                                         self.vr_lhsT[0:2, h, :],
                                         crch[0:2, h, :], start=False, stop=True,
                                         tile_position=(0, hh*64))
                    nc.scalar.copy(self.qa[:, 2+t, n0:n0+NCHUNK], ps[:])

    # ------------------------------------------------------------------
    def p3_proj(self):
        nc, tc = self.nc, self.tc
        f32, bf16 = self.f32, self.bf16
        with tc.tile_pool(name="prj_ps", bufs=4, space="PSUM") as pp, \
             tc.tile_pool(name="prj_sb", bufs=3) as bp:
            for cc in range(32):
                n0 = cc * NCHUNK
                for m in range(4):
                    ps = pp.tile([128, NCHUNK], f32, tag="ps")
                    for k in range(2):
                        nc.tensor.matmul(ps[:], self.w_proj[:, m*2+k],
                                         self.qa[:, 2+k, n0:n0+NCHUNK],
                                         start=(k == 0), stop=(k == 1))
                    osb = bp.tile([128, NCHUNK], bf16, tag="osb")
                    nc.scalar.copy(osb[:], ps[:])
                    nc.sync.dma_start(self.o_out[m, :, n0:n0+NCHUNK], osb[:])


def build_nc(stop_after=None):
    b = Builder(stop_after=stop_after)
    return b.build()


# ----------------------------------------------------------------------------
# SPMD exec wrapper (axon / bass2jax path)
# ----------------------------------------------------------------------------

def _make_exec(nc, n_cores=NCORES, inner=1):
    import jax
    from jax.sharding import Mesh, PartitionSpec
    from jax.experimental.shard_map import shard_map
    from concourse import mybir, bass2jax

    bass2jax.install_neuronx_cc_hook()
    partition_name = nc.partition_id_tensor.name if nc.partition_id_tensor else None

    in_names, out_names, out_avals, zero_outs = [], [], [], []
    for alloc in nc.m.functions[0].allocations:
        if not isinstance(alloc, mybir.MemoryLocationSet):
            continue
        name = alloc.memorylocations[0].name
        if alloc.kind == "ExternalInput":
            if name != partition_name:
                in_names.append(name)
        elif alloc.kind == "ExternalOutput":
            shape = tuple(alloc.tensor_shape)
            dtype = mybir.dt.np(alloc.dtype)
            out_names.append(name)
            out_avals.append(jax.core.ShapedArray(shape, dtype))
            zero_outs.append(np.zeros(shape, dtype))
    n_params = len(in_names)
    all_names = list(in_names) + list(out_names)
    if partition_name is not None:
        all_names.append(partition_name)

    def _body(*args):
        operands = list(args)
        if partition_name is not None:
            operands.append(bass2jax.partition_id_tensor())
        outs = bass2jax._bass_exec_p.bind(
            *operands,
            out_avals=tuple(out_avals),
            in_names=tuple(all_names),
            out_names=tuple(out_names),
            lowering_input_output_aliases=(),
            sim_require_finite=False,
            sim_require_nnan=False,
            nc=nc,
        )
        return tuple(outs)

    def _body_n(*args):
        outs = None
        for _ in range(inner):
            outs = _body(*args)
        return outs

    devices = jax.devices()[:n_cores]
    mesh = Mesh(np.asarray(devices), ("core",))
    nout = len(out_names)
    fn = jax.jit(
        shard_map(_body_n, mesh=mesh,
                  in_specs=(PartitionSpec("core"),) * (n_params + nout),
                  out_specs=(PartitionSpec("core"),) * nout,
                  check_rep=False),
        keep_unused=True,
    )
    return fn, in_names, out_names, out_avals, zero_outs


def _get_state():
    if "fn" not in _STATE:
        nc, _ = build_nc()
        fn, in_names, out_names, out_avals, zero_outs = _make_exec(nc)
        _STATE.update(fn=fn, in_names=in_names, out_names=out_names,
                      out_avals=out_avals, zero_outs=zero_outs, nc=nc)
    return _STATE


def kernel(**inputs):
    import jax
    st = _get_state()
    b = np.asarray(inputs["x"]).shape[0]
    per_core = [build_core_inputs(c, **inputs) for c in range(NCORES)]
    concat_in = [np.concatenate([pc[n] for pc in per_core], axis=0)
                 for n in st["in_names"]]
    concat_zero = [np.zeros((NCORES * z.shape[0], *z.shape[1:]), z.dtype)
                   for z in st["zero_outs"]]
    outs = st["fn"](*concat_in, *concat_zero)
    jax.block_until_ready(outs)
    oidx = st["out_names"].index("out")
    o = np.asarray(outs[oidx]).reshape(NCORES, 4, 128, N).astype(np.float32)
    result = np.empty((b, C, H, W), dtype=np.float32)
    for item in range(b):
        s = o[item*2] + o[item*2+1]
        result[item] = s.reshape(C, H, W)
    return result
